# revision 1
# baseline (speedup 1.0000x reference)
"""MoE layer (top-2 routing, SwiGLU experts) for Trainium2, 8 NeuronCores.

Strategy: expert parallelism — one expert per core. The gate (0.03% of
FLOPs) and the token dispatch/combine run on host; each core runs the
dense SwiGLU FFN for the tokens routed to its expert, in fp32:

  phase 1:  h = silu(x @ w1) * (x @ w3)      (spilled to DRAM, [I, C] blocked)
  phase 2:  y = (h @ w2) * route_weight      (tokens on partitions)

Tokens are gathered per expert on host, padded to a common capacity C
(multiple of 128), and x is passed transposed ([H, C]) so both matmul
phases stream with tokens on the free dimension (phase 1) / stationary
dimension (phase 2) without any on-device transposes.
"""

import os
import sys

for _p in ("/opt/trn_rl_repo", "/root/.axon_site/_ro/trn_rl_repo"):
    if os.path.isdir(_p) and _p not in sys.path:
        sys.path.insert(0, _p)

import numpy as np

import concourse.bass as bass  # noqa: F401  (bass must import before bacc)
import concourse.mybir as mybir
import concourse.tile as tile
from concourse import bacc
from concourse.bass_utils import run_bass_kernel_spmd

H = 1024
E = 8
I = 4096
TOP_K = 2
P = 128
NTOK = 512  # token tile width in phase 1 (fp32 moving-operand max)
F32 = mybir.dt.float32
AF = mybir.ActivationFunctionType

_programs: dict = {}


def build_program(C: int, repeat: int = 1, dtype: str = "f32", tg_size: int = 2, n_inner: bool = True, ph: str = "both", store_coal: bool = False, no_epi: bool = False) -> "bacc.Bacc":
    """One-core SPMD program: SwiGLU FFN for C tokens of one expert.

    repeat > 1 re-emits the whole computation (benchmarking aid: the HW
    time difference between repeat=2 and repeat=1 is one clean iteration).
    dtype: matmul operand precision — "f32" (4-pass, exact), "f32r"
    (FP22 single-ish pass), or "bf16". PSUM accumulation is fp32 always.
    """
    assert C % P == 0
    DT = {"f32": F32, "f32r": mybir.dt.float32r, "bf16": mybir.dt.bfloat16}[dtype]
    Cb = C // P
    HB = H // P  # 8
    IB = I // P  # 32
    NH = H // NTOK  # 2
    # phase-1 token tile widths (512s + one 128-multiple remainder)
    tts = [NTOK] * (C // NTOK)
    if C % NTOK:
        tts.append(C % NTOK)

    nc = bacc.Bacc("TRN2", target_bir_lowering=False, debug=False, num_devices=8)
    x_d = nc.dram_tensor("xt", [H, C], DT, kind="ExternalInput")
    w1_d = nc.dram_tensor("w1", [H, I], DT, kind="ExternalInput")
    w3_d = nc.dram_tensor("w3", [H, I], DT, kind="ExternalInput")
    w2_d = nc.dram_tensor("w2", [I, H], DT, kind="ExternalInput")
    s_d = nc.dram_tensor("st", [P, Cb], F32, kind="ExternalInput")
    # one output region per repeat so no iteration is dead code
    y_d = nc.dram_tensor("y", [repeat * C, H], F32, kind="ExternalOutput")
    # h_act scratch, blocked [c-block, i-block, i-sub(part), c-sub] so that
    # phase-1 writes and phase-2 reads are both contiguous
    ha_d = nc.dram_tensor("hact", [Cb, IB, P, P], DT)
    ha2_d = nc.dram_tensor("hact2", [IB, P, C], DT)  # coalesced-store layout

    xr = x_d.rearrange("(h p) c -> p h c", p=P)  # [128, 8, C]
    w1r = w1_d.rearrange("(h p) i -> p h i", p=P)  # [128, 8, 4096]
    w3r = w3_d.rearrange("(h p) i -> p h i", p=P)
    w2r = w2_d.rearrange("(i p) n -> p i n", p=P)  # [128, 32, 1024]

    with tile.TileContext(nc) as tc:
      for rep in range(repeat):
        # ---------------- phase 1: h = silu(x@w1) * (x@w3) ----------------
        if ph in ("both", "p1"):
         with (
            tc.tile_pool(name=f"xt{rep}", bufs=1) as xt_pool,
            tc.tile_pool(name=f"w13{rep}", bufs=4) as w13_pool,
            tc.tile_pool(name=f"tmp{rep}", bufs=3) as tmp_pool,
            tc.tile_pool(name=f"hst{rep}", bufs=4) as hst_pool,
            tc.tile_pool(name=f"ps1{rep}", bufs=8, space="PSUM") as ps_pool,
        ):
            # x resident, one tile per (h-block, token-tile) for fine deps
            xts = []
            c0 = 0
            for t, w in enumerate(tts):
                row = []
                for h in range(HB):
                    xtile = xt_pool.tile([P, w], DT, tag=f"x{h}_{t}", name=f"x{h}_{t}")
                    nc.sync.dma_start(out=xtile[:], in_=xr[:, h, c0 : c0 + w])
                    row.append(xtile)
                xts.append(row)
                c0 += w

            for i in range(IB):
                w1t = w13_pool.tile([P, HB, P], DT, tag="w13", name=f"w1_{i}")
                w3t = w13_pool.tile([P, HB, P], DT, tag="w13", name=f"w3_{i}")
                nc.sync.dma_start(out=w1t[:], in_=w1r[:, :, i * P : (i + 1) * P])
                nc.sync.dma_start(out=w3t[:], in_=w3r[:, :, i * P : (i + 1) * P])
                # token-tile groups of 2: each weight stationary serves the
                # whole group before switching (amortizes LDWEIGHTS)
                starts = np.cumsum([0] + tts[:-1]).tolist()
                groups = [list(range(g, min(g + tg_size, len(tts))))
                          for g in range(0, len(tts), tg_size)]
                for tg in groups:
                    p1s, p3s = {}, {}
                    for t in tg:
                        p1s[t] = ps_pool.tile([P, NTOK], F32, tag="ps", name=f"p1_{i}_{t}")
                        p3s[t] = ps_pool.tile([P, NTOK], F32, tag="ps", name=f"p3_{i}_{t}")
                    for h in range(HB):
                        for t in tg:
                            nc.tensor.matmul(
                                p1s[t][:, : tts[t]], w1t[:, h, :], xts[t][h][:],
                                start=(h == 0), stop=(h == HB - 1),
                            )
                    for h in range(HB):
                        for t in tg:
                            nc.tensor.matmul(
                                p3s[t][:, : tts[t]], w3t[:, h, :], xts[t][h][:],
                                start=(h == 0), stop=(h == HB - 1),
                            )
                    for t in tg:
                        w, c0, p1, p3 = tts[t], starts[t], p1s[t], p3s[t]
                        hst = hst_pool.tile([P, NTOK], DT, tag="hst", name=f"h_{i}_{t}")
                        if no_epi:
                            nc.scalar.activation(hst[:, :w], p1[:, :w], AF.Copy)
                            nc.scalar.activation(hst[:, :w], p3[:, :w], AF.Copy)
                        else:
                            # silu(p1)*p3 = sigmoid(p1)*p1*p3 (Silu not in CoreSim)
                            tmp = tmp_pool.tile([P, NTOK], F32, tag="tmp", name=f"tmp_{i}_{t}")
                            nc.scalar.activation(tmp[:, :w], p1[:, :w], AF.Sigmoid)
                            t2 = tmp_pool.tile([P, NTOK], F32, tag="tmp2", name=f"t2_{i}_{t}")
                            nc.vector.tensor_mul(t2[:, :w], tmp[:, :w], p1[:, :w])
                            nc.vector.tensor_mul(hst[:, :w], t2[:, :w], p3[:, :w])
                        if store_coal:
                            nc.sync.dma_start(
                                out=ha2_d[i, :, c0 : c0 + w], in_=hst[:, :w]
                            )
                        else:
                            for k in range(w // P):
                                cb = c0 // P + k
                                nc.sync.dma_start(
                                    out=ha_d[cb, i], in_=hst[:, k * P : (k + 1) * P]
                                )

        # ---------------- phase 2: y = (h @ w2) * s ----------------
        if ph in ("both", "p2"):
         with (
            tc.tile_pool(name=f"w2{rep}", bufs=1) as w2_pool,
            tc.tile_pool(name=f"hld{rep}", bufs=3) as hld_pool,
            tc.tile_pool(name=f"sc{rep}", bufs=1) as s_pool,
            tc.tile_pool(name=f"ysb{rep}", bufs=3) as y_pool,
            tc.tile_pool(name=f"ps2{rep}", bufs=4, space="PSUM") as yps_pool,
        ):
            st = s_pool.tile([P, Cb], F32, tag="st", name="st")
            nc.sync.dma_start(out=st[:], in_=s_d[:])
            w2ts = []
            for i in range(IB):
                w2t = w2_pool.tile([P, H], DT, tag=f"w2_{i}", name=f"w2_{i}")
                nc.sync.dma_start(out=w2t[:], in_=w2r[:, i, :])
                w2ts.append(w2t)
            for cb in range(Cb):
                hld = hld_pool.tile([P, IB, P], DT, tag="hld", name=f"hld_{cb}")
                nc.sync.dma_start(out=hld[:], in_=ha_d[cb].rearrange("i p c -> p i c"))
                # n-inner: each hld stationary serves both n halves
                yps = [
                    yps_pool.tile([P, NTOK], F32, tag="yps", name=f"yp_{cb}_{n}")
                    for n in range(NH)
                ]
                if n_inner:
                    for i in range(IB):
                        for n in range(NH):
                            nc.tensor.matmul(
                                yps[n][:],
                                hld[:, i, :],
                                w2ts[i][:, n * NTOK : (n + 1) * NTOK],
                                start=(i == 0), stop=(i == IB - 1),
                            )
                else:
                    for n in range(NH):
                        for i in range(IB):
                            nc.tensor.matmul(
                                yps[n][:],
                                hld[:, i, :],
                                w2ts[i][:, n * NTOK : (n + 1) * NTOK],
                                start=(i == 0), stop=(i == IB - 1),
                            )
                for n in range(NH):
                    ysb = y_pool.tile([P, NTOK], F32, tag="ysb", name=f"y_{cb}_{n}")
                    nc.scalar.activation(
                        ysb[:], yps[n][:], AF.Copy, scale=st[:, cb : cb + 1]
                    )
                    nc.sync.dma_start(
                        out=y_d[
                            rep * C + cb * P : rep * C + (cb + 1) * P,
                            n * NTOK : (n + 1) * NTOK,
                        ],
                        in_=ysb[:],
                    )

    nc.compile()
    return nc


DTYPE = os.environ.get("MOE_DTYPE", "f32")


def get_program(C: int) -> "bacc.Bacc":
    key = (C, DTYPE)
    if key not in _programs:
        _programs[key] = build_program(C, dtype=DTYPE)
    return _programs[key]


def _gate(x: np.ndarray, gate_w: np.ndarray):
    """Top-2 routing, mirroring the jax reference (softmax -> top_k ->
    renormalize). Uses jax for bit-compatible selection when available."""
    try:
        import jax
        import jax.numpy as jnp

        logits = jnp.asarray(x) @ jnp.asarray(gate_w)
        probs = jax.nn.softmax(logits, axis=-1)
        top_vals, top_idx = jax.lax.top_k(probs, TOP_K)
        top_vals = top_vals / jnp.sum(top_vals, axis=-1, keepdims=True)
        return np.asarray(top_vals), np.asarray(top_idx)
    except Exception:
        logits = x @ gate_w
        m = logits.max(-1, keepdims=True)
        p = np.exp(logits - m)
        p /= p.sum(-1, keepdims=True)
        top_idx = np.argsort(-p, axis=-1, kind="stable")[:, :TOP_K]
        top_vals = np.take_along_axis(p, top_idx, axis=-1)
        top_vals = top_vals / top_vals.sum(-1, keepdims=True)
        return top_vals, top_idx


def prepare_dispatch(x, gate_w):
    """Route tokens: per-expert index lists, routing weights, capacity C."""
    top_vals, top_idx = _gate(x, gate_w)
    idxs, wts = [], []
    for e in range(E):
        sel = top_idx == e  # [T, K] bool
        mask = sel.any(axis=-1)
        idx_e = np.nonzero(mask)[0]
        w_e = np.where(sel[idx_e, 0], top_vals[idx_e, 0], top_vals[idx_e, 1])
        idxs.append(idx_e)
        wts.append(w_e.astype(np.float32))
    max_cnt = max(len(ix) for ix in idxs)
    C = max(NTOK, -(-max_cnt // P) * P)
    return idxs, wts, C


def make_in_maps(x, w1, w3, w2, idxs, wts, C, dtype=None):
    dtype = dtype or DTYPE
    if dtype == "bf16":
        import ml_dtypes
        npdt = ml_dtypes.bfloat16
    else:
        npdt = np.float32
    Cb = C // P
    in_maps = []
    for e in range(E):
        cnt = len(idxs[e])
        x_pad = np.zeros((C, H), np.float32)
        x_pad[:cnt] = x[idxs[e]]
        s_pad = np.zeros(C, np.float32)
        s_pad[:cnt] = wts[e]
        in_maps.append(
            {
                "xt": np.ascontiguousarray(x_pad.T).astype(npdt),
                "w1": np.ascontiguousarray(np.asarray(w1[e], dtype=np.float32)).astype(npdt),
                "w3": np.ascontiguousarray(np.asarray(w3[e], dtype=np.float32)).astype(npdt),
                "w2": np.ascontiguousarray(np.asarray(w2[e], dtype=np.float32)).astype(npdt),
                "st": np.ascontiguousarray(s_pad.reshape(Cb, P).T),
            }
        )
    return in_maps


def combine(results, idxs, T):
    out = np.zeros((T, H), np.float32)
    for e in range(E):
        cnt = len(idxs[e])
        out[idxs[e]] += results[e]["y"][:cnt]
    return out


def kernel(hidden_states, gate_w, w1, w3, w2):
    B, S, Hh = hidden_states.shape
    assert Hh == H
    x = np.ascontiguousarray(hidden_states.reshape(-1, H), dtype=np.float32)
    T = x.shape[0]

    idxs, wts, C = prepare_dispatch(x, gate_w)
    nc = get_program(C)
    in_maps = make_in_maps(x, w1, w3, w2, idxs, wts, C)
    res = run_bass_kernel_spmd(nc, in_maps, list(range(E)))
    out = combine(res.results, idxs, T)
    return out.reshape(B, S, H)



# revision 5
# speedup vs baseline: 1.1042x; 1.1042x over previous
"""MoE layer (top-2 routing, SwiGLU experts) for Trainium2, 8 NeuronCores.

Strategy: expert parallelism — one expert per core. The gate (0.03% of
FLOPs) and the token dispatch/combine run on host; each core runs the
dense SwiGLU FFN for the tokens routed to its expert in bf16 (fp32 PSUM
accumulation; rel err ~4e-3 vs the fp32 reference):

  phase 1:  h = silu(x @ w1) * (x @ w3)      (spilled to DRAM, [I-block, p, C])
  phase 2:  y = (h @ w2) * route_weight      (tokens on partitions)

Tokens are gathered per expert on host, padded to a common capacity C
(multiple of 128), and x is passed transposed ([H, C]) so both matmul
phases stream with tokens on the free dimension (phase 1) / stationary
dimension (phase 2) without any on-device transposes.

Schedule notes (DMA dispatch on the issuing engine is serial and costs
~0.5us per 128-row descriptor, so every transfer is laid out as long
contiguous rows and split across the two HWDGE queues, SP + ACT):
  - w1/w3 are passed pre-blocked [IB, 128, H] so each i-tile is one
    contiguous [128, 1024] DMA; 4 i-iterations are prefetched before the
    w2 burst so the PE never starves.
  - h stores go on the ACT queue; everything else on SP.
  - phase 2 reloads h in its stored layout (contiguous rows) and slices
    128-token stationaries out of SBUF; the first token-tile's reloads
    are interleaved into phase 1 so phase 2 starts with zero stall.
"""

import os
import sys

for _p in ("/opt/trn_rl_repo", "/root/.axon_site/_ro/trn_rl_repo"):
    if os.path.isdir(_p) and _p not in sys.path:
        sys.path.insert(0, _p)

import numpy as np

import concourse.bass as bass  # noqa: F401  (bass must import before bacc)
import concourse.mybir as mybir
import concourse.tile as tile
from concourse import bacc
from concourse.bass_utils import run_bass_kernel_spmd

H = 1024
E = 8
I = 4096
TOP_K = 2
P = 128
NTOK = 512  # token tile width in phase 1 (PSUM bank = 512 fp32)
F32 = mybir.dt.float32
AF = mybir.ActivationFunctionType

_programs: dict = {}


def build_program(C: int, repeat: int = 1, dtype: str = "bf16", tg_size: int = 2) -> "bacc.Bacc":
    """One-core SPMD program: SwiGLU FFN for C tokens of one expert.

    repeat > 1 re-emits the whole computation (benchmarking aid: the HW
    time difference between repeat=2 and repeat=1 is one clean iteration).
    dtype: matmul operand precision — "bf16" (default), "f32r" (FP22
    single-pass), or "f32" (4-pass, exact). PSUM accumulation fp32 always.
    """
    assert C % P == 0
    DT = {"f32": F32, "f32r": mybir.dt.float32r, "bf16": mybir.dt.bfloat16}[dtype]
    Cb = C // P
    HB = H // P  # 8
    IB = I // P  # 32
    NH = H // NTOK  # 2
    # phase-1 token tile widths (512s + one 128-multiple remainder)
    tts = [NTOK] * (C // NTOK)
    if C % NTOK:
        tts.append(C % NTOK)
    starts = np.cumsum([0] + tts[:-1]).tolist()
    groups = [list(range(g, min(g + tg_size, len(tts))))
              for g in range(0, len(tts), tg_size)]

    nc = bacc.Bacc("TRN2", target_bir_lowering=False, debug=False, num_devices=8)
    x_d = nc.dram_tensor("xt", [H, C], DT, kind="ExternalInput")
    # w1/w3 pre-blocked on host: [i-block, p, h*128] so one i-tile is a
    # single contiguous [128, 1024] DMA
    w1_d = nc.dram_tensor("w1", [IB, P, H], DT, kind="ExternalInput")
    w3_d = nc.dram_tensor("w3", [IB, P, H], DT, kind="ExternalInput")
    w2_d = nc.dram_tensor("w2", [I, H], DT, kind="ExternalInput")
    s_d = nc.dram_tensor("st", [P, Cb], F32, kind="ExternalInput")
    # one output region per repeat so no iteration is dead code
    y_d = nc.dram_tensor("y", [repeat * C, H], F32, kind="ExternalOutput")
    # h_act scratch, [i-block, i-sub(part), c]: phase-1 stores and phase-2
    # reloads are both contiguous-row DMAs
    ha_d = nc.dram_tensor("hact", [IB, P, C], DT)

    xr = x_d.rearrange("(h p) c -> p h c", p=P)  # [128, 8, C]
    w2r = w2_d.rearrange("(i p) n -> p i n", p=P)  # [128, 32, 1024]

    with tile.TileContext(nc) as tc:
      # pools that span both phases (w2/st/h-reloads prefetched in phase 1)
      with (
          tc.tile_pool(name="w2", bufs=1) as w2_pool,
          tc.tile_pool(name="sc", bufs=1) as s_pool,
          tc.tile_pool(name="hh", bufs=64) as hh_pool,
      ):
       PRE = 4  # i-iterations of w1/w3 prefetched ahead of the w2 burst
       hh_pre: dict = {}
       for rep in range(repeat):
        # ---------------- phase 1: h = silu(x@w1) * (x@w3) ----------------
        with (
            tc.tile_pool(name=f"xt{rep}", bufs=1) as xt_pool,
            tc.tile_pool(name=f"w13{rep}", bufs=2 * PRE) as w13_pool,
            tc.tile_pool(name=f"tmp{rep}", bufs=3) as tmp_pool,
            tc.tile_pool(name=f"hst{rep}", bufs=4) as hst_pool,
            tc.tile_pool(name=f"ps1{rep}", bufs=8, space="PSUM") as ps_pool,
        ):
            # critical path first: i=0 weights + the first token group's x
            # tiles (h-major, t interleaved — the matmul consumption order)
            w13_pre = {}
            w1t = w13_pool.tile([P, HB, P], DT, tag="w13", name=f"w1_0_{rep}")
            w3t = w13_pool.tile([P, HB, P], DT, tag="w13", name=f"w3_0_{rep}")
            nc.sync.dma_start(out=w1t[:], in_=w1_d[0])
            nc.sync.dma_start(out=w3t[:], in_=w3_d[0])
            w13_pre[0] = (w1t, w3t)

            xts = [[None] * HB for _ in tts]

            def load_x(t, h):
                w, c0 = tts[t], starts[t]
                xtile = xt_pool.tile([P, w], DT, tag=f"x{h}_{t}", name=f"x{h}_{t}_{rep}")
                nc.sync.dma_start(out=xtile[:], in_=xr[:, h, c0 : c0 + w])
                xts[t][h] = xtile

            for h in range(HB):
                for t in groups[0]:
                    load_x(t, h)
            for i in range(1, PRE):
                w1t = w13_pool.tile([P, HB, P], DT, tag="w13", name=f"w1_{i}_{rep}")
                w3t = w13_pool.tile([P, HB, P], DT, tag="w13", name=f"w3_{i}_{rep}")
                nc.sync.dma_start(out=w1t[:], in_=w1_d[i])
                nc.sync.dma_start(out=w3t[:], in_=w3_d[i])
                w13_pre[i] = (w1t, w3t)
            for t in range(len(tts)):
                for h in range(HB):
                    if xts[t][h] is None:
                        load_x(t, h)

            # prefetch phase-2 operands during phase 1
            if rep == 0:
                st = s_pool.tile([P, Cb], F32, tag="st", name="st")
                nc.sync.dma_start(out=st[:], in_=s_d[:])
                w2ts = []
                for i in range(IB):
                    w2t = w2_pool.tile([P, H], DT, tag=f"w2_{i}", name=f"w2_{i}")
                    nc.sync.dma_start(out=w2t[:], in_=w2r[:, i, :])
                    w2ts.append(w2t)

            for i in range(IB):
                if i in w13_pre:
                    w1t, w3t = w13_pre.pop(i)
                else:
                    w1t = w13_pool.tile([P, HB, P], DT, tag="w13", name=f"w1_{i}_{rep}")
                    w3t = w13_pool.tile([P, HB, P], DT, tag="w13", name=f"w3_{i}_{rep}")
                    nc.sync.dma_start(out=w1t[:], in_=w1_d[i])
                    nc.sync.dma_start(out=w3t[:], in_=w3_d[i])
                # token-tile groups: each weight stationary serves the whole
                # group before switching (amortizes LDWEIGHTS)
                for tg in groups:
                    p1s, p3s = {}, {}
                    for t in tg:
                        p1s[t] = ps_pool.tile([P, NTOK], F32, tag="ps", name=f"p1_{i}_{t}_{rep}")
                        p3s[t] = ps_pool.tile([P, NTOK], F32, tag="ps", name=f"p3_{i}_{t}_{rep}")
                    for h in range(HB):
                        for t in tg:
                            nc.tensor.matmul(
                                p1s[t][:, : tts[t]], w1t[:, h, :], xts[t][h][:],
                                start=(h == 0), stop=(h == HB - 1),
                            )
                    for h in range(HB):
                        for t in tg:
                            nc.tensor.matmul(
                                p3s[t][:, : tts[t]], w3t[:, h, :], xts[t][h][:],
                                start=(h == 0), stop=(h == HB - 1),
                            )
                    for t in tg:
                        w, c0, p1, p3 = tts[t], starts[t], p1s[t], p3s[t]
                        hst = hst_pool.tile([P, NTOK], DT, tag="hst", name=f"h_{i}_{t}_{rep}")
                        # silu(p1)*p3 = sigmoid(p1)*p1*p3
                        tmp = tmp_pool.tile([P, NTOK], F32, tag="tmp", name=f"tmp_{i}_{t}_{rep}")
                        nc.scalar.activation(tmp[:, :w], p1[:, :w], AF.Sigmoid)
                        t2 = tmp_pool.tile([P, NTOK], F32, tag="tmp2", name=f"t2_{i}_{t}_{rep}")
                        nc.vector.tensor_mul(t2[:, :w], tmp[:, :w], p1[:, :w])
                        nc.vector.tensor_mul(hst[:, :w], t2[:, :w], p3[:, :w])
                        nc.scalar.dma_start(out=ha_d[i, :, c0 : c0 + w], in_=hst[:, :w])
                        # phase-2 prefetch: re-load this i's t=0 slice right
                        # after it lands so phase 2 starts with zero stall
                        if t == 0:
                            hh = hh_pool.tile([P, NTOK], DT, tag="hh", name=f"hh_0_{i}_{rep}")
                            nc.sync.dma_start(out=hh[:], in_=ha_d[i, :, 0:NTOK])
                            hh_pre.setdefault((rep, 0), []).append(hh)

        # ---------------- phase 2: y = (h @ w2) * s ----------------
        with (
            tc.tile_pool(name=f"ysb{rep}", bufs=3) as y_pool,
            tc.tile_pool(name=f"ps2{rep}", bufs=4, space="PSUM") as yps_pool,
        ):
            for t, w in enumerate(tts):
                if (rep, t) in hh_pre:
                    hhs = hh_pre.pop((rep, t))
                else:
                    hhs = []
                    for i in range(IB):
                        hh = hh_pool.tile([P, NTOK], DT, tag="hh", name=f"hh_{t}_{i}_{rep}")
                        nc.sync.dma_start(out=hh[:, :w], in_=ha_d[i, :, starts[t] : starts[t] + w])
                        hhs.append(hh)
                for k in range(w // P):
                    cb = starts[t] // P + k
                    yps = [
                        yps_pool.tile([P, NTOK], F32, tag="yps", name=f"yp_{cb}_{n}_{rep}")
                        for n in range(NH)
                    ]
                    for i in range(IB):
                        for n in range(NH):
                            nc.tensor.matmul(
                                yps[n][:],
                                hhs[i][:, k * P : (k + 1) * P],
                                w2ts[i][:, n * NTOK : (n + 1) * NTOK],
                                start=(i == 0), stop=(i == IB - 1),
                            )
                    for n in range(NH):
                        ysb = y_pool.tile([P, NTOK], F32, tag="ysb", name=f"y_{cb}_{n}_{rep}")
                        nc.scalar.activation(
                            ysb[:], yps[n][:], AF.Copy, scale=st[:, cb : cb + 1]
                        )
                        nc.sync.dma_start(
                            out=y_d[
                                rep * C + cb * P : rep * C + (cb + 1) * P,
                                n * NTOK : (n + 1) * NTOK,
                            ],
                            in_=ysb[:],
                        )

    nc.compile()
    return nc


DTYPE = os.environ.get("MOE_DTYPE", "bf16")


def get_program(C: int) -> "bacc.Bacc":
    key = (C, DTYPE)
    if key not in _programs:
        _programs[key] = build_program(C, dtype=DTYPE)
    return _programs[key]


def _gate(x: np.ndarray, gate_w: np.ndarray):
    """Top-2 routing, mirroring the jax reference (softmax -> top_k ->
    renormalize). Uses jax for bit-compatible selection when available."""
    try:
        import jax
        import jax.numpy as jnp

        logits = jnp.asarray(x) @ jnp.asarray(gate_w)
        probs = jax.nn.softmax(logits, axis=-1)
        top_vals, top_idx = jax.lax.top_k(probs, TOP_K)
        top_vals = top_vals / jnp.sum(top_vals, axis=-1, keepdims=True)
        return np.asarray(top_vals), np.asarray(top_idx)
    except Exception:
        logits = x @ gate_w
        m = logits.max(-1, keepdims=True)
        p = np.exp(logits - m)
        p /= p.sum(-1, keepdims=True)
        top_idx = np.argsort(-p, axis=-1, kind="stable")[:, :TOP_K]
        top_vals = np.take_along_axis(p, top_idx, axis=-1)
        top_vals = top_vals / top_vals.sum(-1, keepdims=True)
        return top_vals, top_idx


def prepare_dispatch(x, gate_w):
    """Route tokens: per-expert index lists, routing weights, capacity C."""
    top_vals, top_idx = _gate(x, gate_w)
    idxs, wts = [], []
    for e in range(E):
        sel = top_idx == e  # [T, K] bool
        mask = sel.any(axis=-1)
        idx_e = np.nonzero(mask)[0]
        w_e = np.where(sel[idx_e, 0], top_vals[idx_e, 0], top_vals[idx_e, 1])
        idxs.append(idx_e)
        wts.append(w_e.astype(np.float32))
    max_cnt = max(len(ix) for ix in idxs)
    C = max(NTOK, -(-max_cnt // P) * P)
    return idxs, wts, C


def _block_w13(w):
    """[H, I] -> [IB, P, H]: w_blocked[i, p, h*P + c] = w[h*P + p, i*P + c]."""
    HB, IB = H // P, I // P
    return np.ascontiguousarray(
        w.reshape(HB, P, IB, P).transpose(2, 1, 0, 3).reshape(IB, P, H)
    )


def make_in_maps(x, w1, w3, w2, idxs, wts, C, dtype=None):
    dtype = dtype or DTYPE
    if dtype == "bf16":
        import ml_dtypes
        npdt = ml_dtypes.bfloat16
    else:
        npdt = np.float32
    Cb = C // P
    in_maps = []
    for e in range(E):
        cnt = len(idxs[e])
        x_pad = np.zeros((C, H), np.float32)
        x_pad[:cnt] = x[idxs[e]]
        s_pad = np.zeros(C, np.float32)
        s_pad[:cnt] = wts[e]
        in_maps.append(
            {
                "xt": np.ascontiguousarray(x_pad.T).astype(npdt),
                "w1": _block_w13(np.asarray(w1[e], dtype=np.float32)).astype(npdt),
                "w3": _block_w13(np.asarray(w3[e], dtype=np.float32)).astype(npdt),
                "w2": np.ascontiguousarray(np.asarray(w2[e], dtype=np.float32)).astype(npdt),
                "st": np.ascontiguousarray(s_pad.reshape(Cb, P).T),
            }
        )
    return in_maps


def combine(results, idxs, T):
    out = np.zeros((T, H), np.float32)
    for e in range(E):
        cnt = len(idxs[e])
        out[idxs[e]] += results[e]["y"][:cnt]
    return out


def kernel(hidden_states, gate_w, w1, w3, w2):
    B, S, Hh = hidden_states.shape
    assert Hh == H
    x = np.ascontiguousarray(hidden_states.reshape(-1, H), dtype=np.float32)
    T = x.shape[0]

    idxs, wts, C = prepare_dispatch(x, gate_w)
    nc = get_program(C)
    in_maps = make_in_maps(x, w1, w3, w2, idxs, wts, C)
    res = run_bass_kernel_spmd(nc, in_maps, list(range(E)))
    out = combine(res.results, idxs, T)
    return out.reshape(B, S, H)


# revision 8
# speedup vs baseline: 1.5266x; 1.3826x over previous
"""MoE layer (top-2 routing, SwiGLU experts) for Trainium2, 8 NeuronCores.

Strategy: expert parallelism — one expert per core. The gate (0.03% of
FLOPs) and the token dispatch/combine run on host; each core runs the
dense SwiGLU FFN for the tokens routed to its expert in bf16 (fp32 PSUM
accumulation; rel err ~4e-3 vs the fp32 reference):

  phase 1:  h = silu(x @ w1) * (x @ w3)      (spilled to DRAM, [I-block, p, C])
  phase 2:  y = (h @ w2) * route_weight      (tokens on partitions)

Tokens are gathered per expert on host, padded to a common capacity C
(multiple of 128), and x is passed transposed ([H, C]) so both matmul
phases stream with tokens on the free dimension (phase 1) / stationary
dimension (phase 2) without any on-device transposes.

Schedule notes (DMA dispatch on the issuing engine is serial and costs
~0.5us per 128-row descriptor, so every transfer is laid out as long
contiguous rows and split across the two HWDGE queues, SP + ACT):
  - w1/w3 are passed pre-blocked [IB, 128, H] so each i-tile is one
    contiguous [128, 1024] DMA; 4 i-iterations are prefetched before the
    w2 burst so the PE never starves.
  - h stores go on the ACT queue; everything else on SP.
  - phase 2 reloads h in its stored layout (contiguous rows) and slices
    128-token stationaries out of SBUF; the first token-tile's reloads
    are interleaved into phase 1 so phase 2 starts with zero stall.
"""

import os
import sys

for _p in ("/opt/trn_rl_repo", "/root/.axon_site/_ro/trn_rl_repo"):
    if os.path.isdir(_p) and _p not in sys.path:
        sys.path.insert(0, _p)

import numpy as np

import concourse.bass as bass  # noqa: F401  (bass must import before bacc)
import concourse.mybir as mybir
import concourse.tile as tile
from concourse import bacc
from concourse.bass_utils import run_bass_kernel_spmd

H = 1024
E = 8
I = 4096
TOP_K = 2
P = 128
NTOK = 512  # token tile width in phase 1 (PSUM bank = 512 fp32)
F32 = mybir.dt.float32
AF = mybir.ActivationFunctionType

_programs: dict = {}


def build_program(C: int, repeat: int = 1, dtype: str = "bf16", tg_size: int = 2) -> "bacc.Bacc":
    """One-core SPMD program: SwiGLU FFN for C tokens of one expert.

    repeat > 1 re-emits the whole computation (benchmarking aid: the HW
    time difference between repeat=2 and repeat=1 is one clean iteration).
    dtype: matmul operand precision — "bf16" (default), "f32r" (FP22
    single-pass), or "f32" (4-pass, exact). PSUM accumulation fp32 always.
    """
    assert C % P == 0
    DT = {"f32": F32, "f32r": mybir.dt.float32r, "bf16": mybir.dt.bfloat16}[dtype]
    Cb = C // P
    HB = H // P  # 8
    IB = I // P  # 32
    NH = H // NTOK  # 2
    # phase-1 token tile widths (512s + one 128-multiple remainder)
    tts = [NTOK] * (C // NTOK)
    if C % NTOK:
        tts.append(C % NTOK)
    starts = np.cumsum([0] + tts[:-1]).tolist()
    groups = [list(range(g, min(g + tg_size, len(tts))))
              for g in range(0, len(tts), tg_size)]

    nc = bacc.Bacc("TRN2", target_bir_lowering=False, debug=False, num_devices=8)
    x_d = nc.dram_tensor("xt", [H, C], DT, kind="ExternalInput")
    # w1/w3 pre-blocked on host: [i-block, p, h*128] so one i-tile is a
    # single contiguous [128, 1024] DMA
    w1_d = nc.dram_tensor("w1", [IB, P, H], DT, kind="ExternalInput")
    w3_d = nc.dram_tensor("w3", [IB, P, H], DT, kind="ExternalInput")
    w2_d = nc.dram_tensor("w2", [I, H], DT, kind="ExternalInput")
    s_d = nc.dram_tensor("st", [P, Cb], F32, kind="ExternalInput")
    # one output region per repeat so no iteration is dead code
    y_d = nc.dram_tensor("y", [repeat * C, H], F32, kind="ExternalOutput")
    # h_act scratch, [i-block, i-sub(part), c]: phase-1 stores and phase-2
    # reloads are both contiguous-row DMAs
    ha_d = nc.dram_tensor("hact", [IB, P, C], DT)

    xr = x_d.rearrange("(h p) c -> p h c", p=P)  # [128, 8, C]
    w2r = w2_d.rearrange("(i p) n -> p i n", p=P)  # [128, 32, 1024]

    with tile.TileContext(nc) as tc:
      # pools that span both phases (w2/st/h-reloads prefetched in phase 1)
      with (
          tc.tile_pool(name="w2", bufs=1) as w2_pool,
          tc.tile_pool(name="sc", bufs=1) as s_pool,
          tc.tile_pool(name="hh", bufs=64) as hh_pool,
      ):
       PRE = 4  # i-iterations of w1/w3 prefetched ahead of the w2 burst
       hh_pre: dict = {}
       for rep in range(repeat):
        # ---------------- phase 1: h = silu(x@w1) * (x@w3) ----------------
        with (
            tc.tile_pool(name=f"xt{rep}", bufs=1) as xt_pool,
            tc.tile_pool(name=f"w13{rep}", bufs=2 * PRE) as w13_pool,
            tc.tile_pool(name=f"tmp{rep}", bufs=3) as tmp_pool,
            tc.tile_pool(name=f"hst{rep}", bufs=4) as hst_pool,
            tc.tile_pool(name=f"ps1{rep}", bufs=8, space="PSUM") as ps_pool,
        ):
            # critical path first: i=0 weights + the first token groups' x
            # tiles (h-major, t interleaved — the matmul consumption order).
            # Startup dispatch alternates SP/ACT queues: DMA issue is serial
            # per engine (~0.5us/128-row descriptor), so two queues double
            # the supply rate while the PE ramps.
            _eng = [nc.sync, nc.scalar]
            _ec = [0]

            def eng():
                _ec[0] += 1
                return _eng[_ec[0] % 2]

            w13_pre = {}
            w1t = w13_pool.tile([P, HB, P], DT, tag="w13", name=f"w1_0_{rep}")
            w3t = w13_pool.tile([P, HB, P], DT, tag="w13", name=f"w3_0_{rep}")
            eng().dma_start(out=w1t[:], in_=w1_d[0])
            eng().dma_start(out=w3t[:], in_=w3_d[0])
            w13_pre[0] = (w1t, w3t)

            xts = [[None] * HB for _ in tts]

            def load_x(t, h, e=None):
                w, c0 = tts[t], starts[t]
                xtile = xt_pool.tile([P, w], DT, tag=f"x{h}_{t}", name=f"x{h}_{t}_{rep}")
                (e or nc.sync).dma_start(out=xtile[:], in_=xr[:, h, c0 : c0 + w])
                xts[t][h] = xtile

            early_t = groups[0] + (groups[1] if len(groups) > 1 else [])
            for h in range(HB):
                for t in early_t:
                    load_x(t, h, eng())
            for i in range(1, PRE):
                w1t = w13_pool.tile([P, HB, P], DT, tag="w13", name=f"w1_{i}_{rep}")
                w3t = w13_pool.tile([P, HB, P], DT, tag="w13", name=f"w3_{i}_{rep}")
                eng().dma_start(out=w1t[:], in_=w1_d[i])
                eng().dma_start(out=w3t[:], in_=w3_d[i])
                w13_pre[i] = (w1t, w3t)
            for t in range(len(tts)):
                for h in range(HB):
                    if xts[t][h] is None:
                        load_x(t, h)

            # prefetch phase-2 operands during phase 1
            if rep == 0:
                st = s_pool.tile([P, Cb], F32, tag="st", name="st")
                nc.sync.dma_start(out=st[:], in_=s_d[:])
                w2ts = []
                for i in range(IB):
                    w2t = w2_pool.tile([P, H], DT, tag=f"w2_{i}", name=f"w2_{i}")
                    nc.sync.dma_start(out=w2t[:], in_=w2r[:, i, :])
                    w2ts.append(w2t)

            for i in range(IB):
                if i in w13_pre:
                    w1t, w3t = w13_pre.pop(i)
                else:
                    w1t = w13_pool.tile([P, HB, P], DT, tag="w13", name=f"w1_{i}_{rep}")
                    w3t = w13_pool.tile([P, HB, P], DT, tag="w13", name=f"w3_{i}_{rep}")
                    nc.sync.dma_start(out=w1t[:], in_=w1_d[i])
                    nc.sync.dma_start(out=w3t[:], in_=w3_d[i])
                # token-tile groups: each weight stationary serves the whole
                # group before switching (amortizes LDWEIGHTS)
                for tg in groups:
                    p1s, p3s = {}, {}
                    for t in tg:
                        p1s[t] = ps_pool.tile([P, NTOK], F32, tag="ps", name=f"p1_{i}_{t}_{rep}")
                        p3s[t] = ps_pool.tile([P, NTOK], F32, tag="ps", name=f"p3_{i}_{t}_{rep}")
                    for h in range(HB):
                        for t in tg:
                            nc.tensor.matmul(
                                p1s[t][:, : tts[t]], w1t[:, h, :], xts[t][h][:],
                                start=(h == 0), stop=(h == HB - 1),
                            )
                    for h in range(HB):
                        for t in tg:
                            nc.tensor.matmul(
                                p3s[t][:, : tts[t]], w3t[:, h, :], xts[t][h][:],
                                start=(h == 0), stop=(h == HB - 1),
                            )
                    for t in tg:
                        w, c0, p1, p3 = tts[t], starts[t], p1s[t], p3s[t]
                        hst = hst_pool.tile([P, NTOK], DT, tag="hst", name=f"h_{i}_{t}_{rep}")
                        # silu(p1)*p3 = sigmoid(p1)*p1*p3
                        tmp = tmp_pool.tile([P, NTOK], F32, tag="tmp", name=f"tmp_{i}_{t}_{rep}")
                        nc.scalar.activation(tmp[:, :w], p1[:, :w], AF.Sigmoid)
                        t2 = tmp_pool.tile([P, NTOK], F32, tag="tmp2", name=f"t2_{i}_{t}_{rep}")
                        nc.vector.tensor_mul(t2[:, :w], tmp[:, :w], p1[:, :w])
                        nc.vector.tensor_mul(hst[:, :w], t2[:, :w], p3[:, :w])
                        nc.scalar.dma_start(out=ha_d[i, :, c0 : c0 + w], in_=hst[:, :w])
                        # phase-2 prefetch: re-load this i's t=0 slice right
                        # after it lands so phase 2 starts with zero stall
                        if t == 0:
                            hh = hh_pool.tile([P, NTOK], DT, tag="hh", name=f"hh_0_{i}_{rep}")
                            nc.sync.dma_start(out=hh[:], in_=ha_d[i, :, 0:NTOK])
                            hh_pre.setdefault((rep, 0), []).append(hh)

        # ---------------- phase 2: y = (h @ w2) * s ----------------
        with (
            tc.tile_pool(name=f"ysb{rep}", bufs=3) as y_pool,
            tc.tile_pool(name=f"ps2{rep}", bufs=4, space="PSUM") as yps_pool,
        ):
            for t, w in enumerate(tts):
                if (rep, t) in hh_pre:
                    hhs = hh_pre.pop((rep, t))
                else:
                    hhs = []
                    for i in range(IB):
                        hh = hh_pool.tile([P, NTOK], DT, tag="hh", name=f"hh_{t}_{i}_{rep}")
                        nc.sync.dma_start(out=hh[:, :w], in_=ha_d[i, :, starts[t] : starts[t] + w])
                        hhs.append(hh)
                for k in range(w // P):
                    cb = starts[t] // P + k
                    yps = [
                        yps_pool.tile([P, NTOK], F32, tag="yps", name=f"yp_{cb}_{n}_{rep}")
                        for n in range(NH)
                    ]
                    for i in range(IB):
                        for n in range(NH):
                            nc.tensor.matmul(
                                yps[n][:],
                                hhs[i][:, k * P : (k + 1) * P],
                                w2ts[i][:, n * NTOK : (n + 1) * NTOK],
                                start=(i == 0), stop=(i == IB - 1),
                            )
                    for n in range(NH):
                        ysb = y_pool.tile([P, NTOK], F32, tag="ysb", name=f"y_{cb}_{n}_{rep}")
                        nc.scalar.activation(
                            ysb[:], yps[n][:], AF.Copy, scale=st[:, cb : cb + 1]
                        )
                        nc.sync.dma_start(
                            out=y_d[
                                rep * C + cb * P : rep * C + (cb + 1) * P,
                                n * NTOK : (n + 1) * NTOK,
                            ],
                            in_=ysb[:],
                        )

    nc.compile()
    return nc


DTYPE = os.environ.get("MOE_DTYPE", "bf16")


def get_program(C: int) -> "bacc.Bacc":
    key = (C, DTYPE)
    if key not in _programs:
        _programs[key] = build_program(C, dtype=DTYPE)
    return _programs[key]


def _gate(x: np.ndarray, gate_w: np.ndarray):
    """Top-2 routing, mirroring the jax reference (softmax -> top_k ->
    renormalize). Uses jax for bit-compatible selection when available."""
    try:
        import jax
        import jax.numpy as jnp

        logits = jnp.asarray(x) @ jnp.asarray(gate_w)
        probs = jax.nn.softmax(logits, axis=-1)
        top_vals, top_idx = jax.lax.top_k(probs, TOP_K)
        top_vals = top_vals / jnp.sum(top_vals, axis=-1, keepdims=True)
        return np.asarray(top_vals), np.asarray(top_idx)
    except Exception:
        logits = x @ gate_w
        m = logits.max(-1, keepdims=True)
        p = np.exp(logits - m)
        p /= p.sum(-1, keepdims=True)
        top_idx = np.argsort(-p, axis=-1, kind="stable")[:, :TOP_K]
        top_vals = np.take_along_axis(p, top_idx, axis=-1)
        top_vals = top_vals / top_vals.sum(-1, keepdims=True)
        return top_vals, top_idx


OVF_MAX = 32  # max token-expert pairs computed on host to save one C block


def prepare_dispatch_v2(x, gate_w):
    """Route tokens: per-expert index lists, routing weights, capacity C.

    If only a few tokens push the max expert count over a 128 boundary,
    shrink the device capacity by one block and return those tokens as
    host-overflow work ([(e, idx_array, wt_array), ...]) — every core then
    runs one fewer token block.
    """
    top_vals, top_idx = _gate(x, gate_w)
    idxs, wts = [], []
    for e in range(E):
        sel = top_idx == e  # [T, K] bool
        mask = sel.any(axis=-1)
        idx_e = np.nonzero(mask)[0]
        w_e = np.where(sel[idx_e, 0], top_vals[idx_e, 0], top_vals[idx_e, 1])
        idxs.append(idx_e)
        wts.append(w_e.astype(np.float32))
    max_cnt = max(len(ix) for ix in idxs)
    C = max(NTOK, -(-max_cnt // P) * P)
    ovf = []
    C1 = C - P
    if C1 >= NTOK:
        n_over = sum(max(0, len(ix) - C1) for ix in idxs)
        if 0 < n_over <= OVF_MAX:
            for e in range(E):
                if len(idxs[e]) > C1:
                    ovf.append((e, idxs[e][C1:], wts[e][C1:]))
                    idxs[e] = idxs[e][:C1]
                    wts[e] = wts[e][:C1]
            C = C1
    return idxs, wts, C, ovf


def prepare_dispatch(x, gate_w):
    idxs, wts, C, _ = prepare_dispatch_v2(x, gate_w)
    return idxs, wts, C


def _block_w13(w):
    """[H, I] -> [IB, P, H]: w_blocked[i, p, h*P + c] = w[h*P + p, i*P + c]."""
    HB, IB = H // P, I // P
    return np.ascontiguousarray(
        w.reshape(HB, P, IB, P).transpose(2, 1, 0, 3).reshape(IB, P, H)
    )


def make_in_maps(x, w1, w3, w2, idxs, wts, C, dtype=None):
    dtype = dtype or DTYPE
    if dtype == "bf16":
        import ml_dtypes
        npdt = ml_dtypes.bfloat16
    else:
        npdt = np.float32
    Cb = C // P
    in_maps = []
    for e in range(E):
        cnt = len(idxs[e])
        x_pad = np.zeros((C, H), np.float32)
        x_pad[:cnt] = x[idxs[e]]
        s_pad = np.zeros(C, np.float32)
        s_pad[:cnt] = wts[e]
        in_maps.append(
            {
                "xt": np.ascontiguousarray(x_pad.T).astype(npdt),
                "w1": _block_w13(np.asarray(w1[e], dtype=np.float32)).astype(npdt),
                "w3": _block_w13(np.asarray(w3[e], dtype=np.float32)).astype(npdt),
                "w2": np.ascontiguousarray(np.asarray(w2[e], dtype=np.float32)).astype(npdt),
                "st": np.ascontiguousarray(s_pad.reshape(Cb, P).T),
            }
        )
    return in_maps


def combine(results, idxs, T):
    out = np.zeros((T, H), np.float32)
    for e in range(E):
        cnt = len(idxs[e])
        out[idxs[e]] += results[e]["y"][:cnt]
    return out


def kernel(hidden_states, gate_w, w1, w3, w2):
    B, S, Hh = hidden_states.shape
    assert Hh == H
    x = np.ascontiguousarray(hidden_states.reshape(-1, H), dtype=np.float32)
    T = x.shape[0]

    idxs, wts, C, ovf = prepare_dispatch_v2(x, gate_w)
    nc = get_program(C)
    in_maps = make_in_maps(x, w1, w3, w2, idxs, wts, C)
    res = run_bass_kernel_spmd(nc, in_maps, list(range(E)))
    out = combine(res.results, idxs, T)
    for e, t_idx, t_w in ovf:  # host-side capacity-overflow tokens
        xe = x[t_idx]
        w1e = np.asarray(w1[e], np.float32)
        w3e = np.asarray(w3[e], np.float32)
        w2e = np.asarray(w2[e], np.float32)
        a = xe @ w1e
        h = (a / (1.0 + np.exp(-a))) * (xe @ w3e)
        out[t_idx] += (h @ w2e) * t_w[:, None]
    return out.reshape(B, S, H)


# revision 9
# speedup vs baseline: 1.6341x; 1.0704x over previous
"""MoE layer (top-2 routing, SwiGLU experts) for Trainium2, 8 NeuronCores.

Strategy: expert parallelism — one expert per core. The gate (0.03% of
FLOPs) and the token dispatch/combine run on host; each core runs the
dense SwiGLU FFN for the tokens routed to its expert in bf16 (fp32 PSUM
accumulation; rel err ~4e-3 vs the fp32 reference):

  phase 1:  h = silu(x @ w1) * (x @ w3)      (spilled to DRAM, [I-block, p, C])
  phase 2:  y = (h @ w2) * route_weight      (tokens on partitions)

Tokens are gathered per expert on host, padded to a common capacity C
(multiple of 128), and x is passed transposed ([H, C]) so both matmul
phases stream with tokens on the free dimension (phase 1) / stationary
dimension (phase 2) without any on-device transposes.

Schedule notes (DMA dispatch on the issuing engine is serial and costs
~0.5us per 128-row descriptor, so every transfer is laid out as long
contiguous rows and split across the two HWDGE queues, SP + ACT):
  - w1/w3 are passed pre-blocked [IB, 128, H] so each i-tile is one
    contiguous [128, 1024] DMA; 4 i-iterations are prefetched before the
    w2 burst so the PE never starves.
  - h stores go on the ACT queue; everything else on SP.
  - phase 2 reloads h in its stored layout (contiguous rows) and slices
    128-token stationaries out of SBUF; the first token-tile's reloads
    are interleaved into phase 1 so phase 2 starts with zero stall.
"""

import os
import sys

for _p in ("/opt/trn_rl_repo", "/root/.axon_site/_ro/trn_rl_repo"):
    if os.path.isdir(_p) and _p not in sys.path:
        sys.path.insert(0, _p)

import numpy as np

import concourse.bass as bass  # noqa: F401  (bass must import before bacc)
import concourse.mybir as mybir
import concourse.tile as tile
from concourse import bacc
from concourse.bass_utils import run_bass_kernel_spmd

H = 1024
E = 8
I = 4096
TOP_K = 2
P = 128
NTOK = 512  # token tile width in phase 1 (PSUM bank = 512 fp32)
F32 = mybir.dt.float32
AF = mybir.ActivationFunctionType

_programs: dict = {}


def build_program(C: int, repeat: int = 1, dtype: str = "bf16", tg_size: int = 2) -> "bacc.Bacc":
    """One-core SPMD program: SwiGLU FFN for C tokens of one expert.

    repeat > 1 re-emits the whole computation (benchmarking aid: the HW
    time difference between repeat=2 and repeat=1 is one clean iteration).
    dtype: matmul operand precision — "bf16" (default), "f32r" (FP22
    single-pass), or "f32" (4-pass, exact). PSUM accumulation fp32 always.
    """
    assert C % P == 0
    DT = {"f32": F32, "f32r": mybir.dt.float32r, "bf16": mybir.dt.bfloat16}[dtype]
    Cb = C // P
    HB = H // P  # 8
    IB = I // P  # 32
    NH = H // NTOK  # 2
    # phase-1 token tile widths (512s + one 128-multiple remainder)
    tts = [NTOK] * (C // NTOK)
    if C % NTOK:
        tts.append(C % NTOK)
    starts = np.cumsum([0] + tts[:-1]).tolist()
    groups = [list(range(g, min(g + tg_size, len(tts))))
              for g in range(0, len(tts), tg_size)]

    nc = bacc.Bacc("TRN2", target_bir_lowering=False, debug=False, num_devices=8)
    x_d = nc.dram_tensor("xt", [H, C], DT, kind="ExternalInput")
    # w1/w3 pre-blocked on host: [i-block, p, h*128] so one i-tile is a
    # single contiguous [128, 1024] DMA
    w1_d = nc.dram_tensor("w1", [IB, P, H], DT, kind="ExternalInput")
    w3_d = nc.dram_tensor("w3", [IB, P, H], DT, kind="ExternalInput")
    w2_d = nc.dram_tensor("w2", [I, H], DT, kind="ExternalInput")
    s_d = nc.dram_tensor("st", [P, Cb], F32, kind="ExternalInput")
    # one output region per repeat so no iteration is dead code
    y_d = nc.dram_tensor("y", [repeat * C, H], F32, kind="ExternalOutput")
    # h_act scratch, [i-block, i-sub(part), c]: phase-1 stores and phase-2
    # reloads are both contiguous-row DMAs
    ha_d = nc.dram_tensor("hact", [IB, P, C], DT)

    xr = x_d.rearrange("(h p) c -> p h c", p=P)  # [128, 8, C]
    w2r = w2_d.rearrange("(i p) n -> p i n", p=P)  # [128, 32, 1024]

    with tile.TileContext(nc) as tc:
      # pools that span both phases (w2/st/h-reloads prefetched in phase 1)
      with (
          tc.tile_pool(name="w2", bufs=1) as w2_pool,
          tc.tile_pool(name="sc", bufs=1) as s_pool,
          tc.tile_pool(name="hh", bufs=64) as hh_pool,
      ):
       PRE = 4  # i-iterations of w1/w3 prefetched ahead of the w2 burst
       hh_pre: dict = {}
       for rep in range(repeat):
        # ---------------- phase 1: h = silu(x@w1) * (x@w3) ----------------
        with (
            tc.tile_pool(name=f"xt{rep}", bufs=1) as xt_pool,
            tc.tile_pool(name=f"w13{rep}", bufs=2 * PRE) as w13_pool,
            tc.tile_pool(name=f"tmp{rep}", bufs=3) as tmp_pool,
            tc.tile_pool(name=f"hst{rep}", bufs=4) as hst_pool,
            tc.tile_pool(name=f"ps1{rep}", bufs=8, space="PSUM") as ps_pool,
        ):
            # critical path first: i=0 weights + the first token groups' x
            # tiles (h-major, t interleaved — the matmul consumption order).
            # Startup dispatch alternates SP/ACT queues: DMA issue is serial
            # per engine (~0.5us/128-row descriptor), so two queues double
            # the supply rate while the PE ramps.
            _eng = [nc.sync, nc.scalar]
            _ec = [0]

            def eng():
                _ec[0] += 1
                return _eng[_ec[0] % 2]

            w13_pre = {}
            w1t = w13_pool.tile([P, HB, P], DT, tag="w13", name=f"w1_0_{rep}")
            w3t = w13_pool.tile([P, HB, P], DT, tag="w13", name=f"w3_0_{rep}")
            eng().dma_start(out=w1t[:], in_=w1_d[0])
            eng().dma_start(out=w3t[:], in_=w3_d[0])
            w13_pre[0] = (w1t, w3t)

            xts = [[None] * HB for _ in tts]

            def load_x(t, h, e=None):
                w, c0 = tts[t], starts[t]
                xtile = xt_pool.tile([P, w], DT, tag=f"x{h}_{t}", name=f"x{h}_{t}_{rep}")
                (e or nc.sync).dma_start(out=xtile[:], in_=xr[:, h, c0 : c0 + w])
                xts[t][h] = xtile

            early_t = groups[0] + (groups[1] if len(groups) > 1 else [])
            for h in range(HB):
                for t in early_t:
                    load_x(t, h, eng())
            for i in range(1, PRE):
                w1t = w13_pool.tile([P, HB, P], DT, tag="w13", name=f"w1_{i}_{rep}")
                w3t = w13_pool.tile([P, HB, P], DT, tag="w13", name=f"w3_{i}_{rep}")
                eng().dma_start(out=w1t[:], in_=w1_d[i])
                eng().dma_start(out=w3t[:], in_=w3_d[i])
                w13_pre[i] = (w1t, w3t)
            for t in range(len(tts)):
                for h in range(HB):
                    if xts[t][h] is None:
                        load_x(t, h)

            # prefetch phase-2 operands during phase 1
            if rep == 0:
                st = s_pool.tile([P, Cb], F32, tag="st", name="st")
                nc.sync.dma_start(out=st[:], in_=s_d[:])
                w2ts = []
                for i in range(IB):
                    w2t = w2_pool.tile([P, H], DT, tag=f"w2_{i}", name=f"w2_{i}")
                    nc.sync.dma_start(out=w2t[:], in_=w2r[:, i, :])
                    w2ts.append(w2t)

            for i in range(IB):
                if i in w13_pre:
                    w1t, w3t = w13_pre.pop(i)
                else:
                    w1t = w13_pool.tile([P, HB, P], DT, tag="w13", name=f"w1_{i}_{rep}")
                    w3t = w13_pool.tile([P, HB, P], DT, tag="w13", name=f"w3_{i}_{rep}")
                    nc.sync.dma_start(out=w1t[:], in_=w1_d[i])
                    nc.sync.dma_start(out=w3t[:], in_=w3_d[i])
                # token-tile groups: each weight stationary serves the whole
                # group before switching (amortizes LDWEIGHTS)
                for tg in groups:
                    p1s, p3s = {}, {}
                    for t in tg:
                        p1s[t] = ps_pool.tile([P, NTOK], F32, tag="ps", name=f"p1_{i}_{t}_{rep}")
                        p3s[t] = ps_pool.tile([P, NTOK], F32, tag="ps", name=f"p3_{i}_{t}_{rep}")
                    for h in range(HB):
                        for t in tg:
                            nc.tensor.matmul(
                                p1s[t][:, : tts[t]], w1t[:, h, :], xts[t][h][:],
                                start=(h == 0), stop=(h == HB - 1),
                            )
                    for h in range(HB):
                        for t in tg:
                            nc.tensor.matmul(
                                p3s[t][:, : tts[t]], w3t[:, h, :], xts[t][h][:],
                                start=(h == 0), stop=(h == HB - 1),
                            )
                    for t in tg:
                        w, c0, p1, p3 = tts[t], starts[t], p1s[t], p3s[t]
                        hst = hst_pool.tile([P, NTOK], DT, tag="hst", name=f"h_{i}_{t}_{rep}")
                        tmp = tmp_pool.tile([P, NTOK], F32, tag="tmp", name=f"tmp_{i}_{t}_{rep}")
                        nc.scalar.activation(tmp[:, :w], p1[:, :w], AF.Silu)
                        nc.vector.tensor_mul(hst[:, :w], tmp[:, :w], p3[:, :w])
                        nc.scalar.dma_start(out=ha_d[i, :, c0 : c0 + w], in_=hst[:, :w])
                        # phase-2 prefetch: re-load this i's t=0 slice right
                        # after it lands so phase 2 starts with zero stall
                        if t == 0:
                            hh = hh_pool.tile([P, NTOK], DT, tag="hh", name=f"hh_0_{i}_{rep}")
                            nc.sync.dma_start(out=hh[:], in_=ha_d[i, :, 0:NTOK])
                            hh_pre.setdefault((rep, 0), []).append(hh)

        # ---------------- phase 2: y = (h @ w2) * s ----------------
        with (
            tc.tile_pool(name=f"ysb{rep}", bufs=3) as y_pool,
            tc.tile_pool(name=f"ps2{rep}", bufs=4, space="PSUM") as yps_pool,
        ):
            for t, w in enumerate(tts):
                if (rep, t) in hh_pre:
                    hhs = hh_pre.pop((rep, t))
                else:
                    hhs = []
                    for i in range(IB):
                        hh = hh_pool.tile([P, NTOK], DT, tag="hh", name=f"hh_{t}_{i}_{rep}")
                        nc.sync.dma_start(out=hh[:, :w], in_=ha_d[i, :, starts[t] : starts[t] + w])
                        hhs.append(hh)
                for k in range(w // P):
                    cb = starts[t] // P + k
                    yps = [
                        yps_pool.tile([P, NTOK], F32, tag="yps", name=f"yp_{cb}_{n}_{rep}")
                        for n in range(NH)
                    ]
                    for i in range(IB):
                        for n in range(NH):
                            nc.tensor.matmul(
                                yps[n][:],
                                hhs[i][:, k * P : (k + 1) * P],
                                w2ts[i][:, n * NTOK : (n + 1) * NTOK],
                                start=(i == 0), stop=(i == IB - 1),
                            )
                    for n in range(NH):
                        ysb = y_pool.tile([P, NTOK], F32, tag="ysb", name=f"y_{cb}_{n}_{rep}")
                        nc.scalar.activation(
                            ysb[:], yps[n][:], AF.Copy, scale=st[:, cb : cb + 1]
                        )
                        nc.sync.dma_start(
                            out=y_d[
                                rep * C + cb * P : rep * C + (cb + 1) * P,
                                n * NTOK : (n + 1) * NTOK,
                            ],
                            in_=ysb[:],
                        )

    nc.compile()
    return nc


DTYPE = os.environ.get("MOE_DTYPE", "bf16")


def get_program(C: int) -> "bacc.Bacc":
    key = (C, DTYPE)
    if key not in _programs:
        _programs[key] = build_program(C, dtype=DTYPE)
    return _programs[key]


def _gate(x: np.ndarray, gate_w: np.ndarray):
    """Top-2 routing, mirroring the jax reference (softmax -> top_k ->
    renormalize). Uses jax for bit-compatible selection when available."""
    try:
        import jax
        import jax.numpy as jnp

        logits = jnp.asarray(x) @ jnp.asarray(gate_w)
        probs = jax.nn.softmax(logits, axis=-1)
        top_vals, top_idx = jax.lax.top_k(probs, TOP_K)
        top_vals = top_vals / jnp.sum(top_vals, axis=-1, keepdims=True)
        return np.asarray(top_vals), np.asarray(top_idx)
    except Exception:
        logits = x @ gate_w
        m = logits.max(-1, keepdims=True)
        p = np.exp(logits - m)
        p /= p.sum(-1, keepdims=True)
        top_idx = np.argsort(-p, axis=-1, kind="stable")[:, :TOP_K]
        top_vals = np.take_along_axis(p, top_idx, axis=-1)
        top_vals = top_vals / top_vals.sum(-1, keepdims=True)
        return top_vals, top_idx


OVF_MAX = 32  # max token-expert pairs computed on host to save one C block


def prepare_dispatch_v2(x, gate_w):
    """Route tokens: per-expert index lists, routing weights, capacity C.

    If only a few tokens push the max expert count over a 128 boundary,
    shrink the device capacity by one block and return those tokens as
    host-overflow work ([(e, idx_array, wt_array), ...]) — every core then
    runs one fewer token block.
    """
    top_vals, top_idx = _gate(x, gate_w)
    idxs, wts = [], []
    for e in range(E):
        sel = top_idx == e  # [T, K] bool
        mask = sel.any(axis=-1)
        idx_e = np.nonzero(mask)[0]
        w_e = np.where(sel[idx_e, 0], top_vals[idx_e, 0], top_vals[idx_e, 1])
        idxs.append(idx_e)
        wts.append(w_e.astype(np.float32))
    max_cnt = max(len(ix) for ix in idxs)
    C = max(NTOK, -(-max_cnt // P) * P)
    ovf = []
    C1 = C - P
    if C1 >= NTOK:
        n_over = sum(max(0, len(ix) - C1) for ix in idxs)
        if 0 < n_over <= OVF_MAX:
            for e in range(E):
                if len(idxs[e]) > C1:
                    ovf.append((e, idxs[e][C1:], wts[e][C1:]))
                    idxs[e] = idxs[e][:C1]
                    wts[e] = wts[e][:C1]
            C = C1
    return idxs, wts, C, ovf


def prepare_dispatch(x, gate_w):
    idxs, wts, C, _ = prepare_dispatch_v2(x, gate_w)
    return idxs, wts, C


def _block_w13(w):
    """[H, I] -> [IB, P, H]: w_blocked[i, p, h*P + c] = w[h*P + p, i*P + c]."""
    HB, IB = H // P, I // P
    return np.ascontiguousarray(
        w.reshape(HB, P, IB, P).transpose(2, 1, 0, 3).reshape(IB, P, H)
    )


def make_in_maps(x, w1, w3, w2, idxs, wts, C, dtype=None):
    dtype = dtype or DTYPE
    if dtype == "bf16":
        import ml_dtypes
        npdt = ml_dtypes.bfloat16
    else:
        npdt = np.float32
    Cb = C // P
    in_maps = []
    for e in range(E):
        cnt = len(idxs[e])
        x_pad = np.zeros((C, H), np.float32)
        x_pad[:cnt] = x[idxs[e]]
        s_pad = np.zeros(C, np.float32)
        s_pad[:cnt] = wts[e]
        in_maps.append(
            {
                "xt": np.ascontiguousarray(x_pad.T).astype(npdt),
                "w1": _block_w13(np.asarray(w1[e], dtype=np.float32)).astype(npdt),
                "w3": _block_w13(np.asarray(w3[e], dtype=np.float32)).astype(npdt),
                "w2": np.ascontiguousarray(np.asarray(w2[e], dtype=np.float32)).astype(npdt),
                "st": np.ascontiguousarray(s_pad.reshape(Cb, P).T),
            }
        )
    return in_maps


def combine(results, idxs, T):
    out = np.zeros((T, H), np.float32)
    for e in range(E):
        cnt = len(idxs[e])
        out[idxs[e]] += results[e]["y"][:cnt]
    return out


def kernel(hidden_states, gate_w, w1, w3, w2):
    B, S, Hh = hidden_states.shape
    assert Hh == H
    x = np.ascontiguousarray(hidden_states.reshape(-1, H), dtype=np.float32)
    T = x.shape[0]

    idxs, wts, C, ovf = prepare_dispatch_v2(x, gate_w)
    nc = get_program(C)
    in_maps = make_in_maps(x, w1, w3, w2, idxs, wts, C)
    res = run_bass_kernel_spmd(nc, in_maps, list(range(E)))
    out = combine(res.results, idxs, T)
    for e, t_idx, t_w in ovf:  # host-side capacity-overflow tokens
        xe = x[t_idx]
        w1e = np.asarray(w1[e], np.float32)
        w3e = np.asarray(w3[e], np.float32)
        w2e = np.asarray(w2[e], np.float32)
        a = xe @ w1e
        h = (a / (1.0 + np.exp(-a))) * (xe @ w3e)
        out[t_idx] += (h @ w2e) * t_w[:, None]
    return out.reshape(B, S, H)


# revision 10
# speedup vs baseline: 1.7849x; 1.0923x over previous
"""MoE layer (top-2 routing, SwiGLU experts) for Trainium2, 8 NeuronCores.

Strategy: expert parallelism — one expert per core. The gate (0.03% of
FLOPs) and the token dispatch/combine run on host; each core runs the
dense SwiGLU FFN for the tokens routed to its expert in bf16 (fp32 PSUM
accumulation; rel err ~4e-3 vs the fp32 reference):

  phase 1:  h = silu(x @ w1) * (x @ w3)      (spilled to DRAM, [I-block, p, C])
  phase 2:  y = (h @ w2) * route_weight      (tokens on partitions)

Tokens are gathered per expert on host, padded to a common capacity C
(multiple of 128), and x is passed transposed ([H, C]) so both matmul
phases stream with tokens on the free dimension (phase 1) / stationary
dimension (phase 2) without any on-device transposes.

Schedule notes (DMA dispatch on the issuing engine is serial and costs
~0.5us per 128-row descriptor, so every transfer is laid out as long
contiguous rows and split across the two HWDGE queues, SP + ACT):
  - w1/w3 are passed pre-blocked [IB, 128, H] so each i-tile is one
    contiguous [128, 1024] DMA; 4 i-iterations are prefetched before the
    w2 burst so the PE never starves.
  - h stores go on the ACT queue; everything else on SP.
  - phase 2 reloads h in its stored layout (contiguous rows) and slices
    128-token stationaries out of SBUF; the first token-tile's reloads
    are interleaved into phase 1 so phase 2 starts with zero stall.
"""

import os
import sys

for _p in ("/opt/trn_rl_repo", "/root/.axon_site/_ro/trn_rl_repo"):
    if os.path.isdir(_p) and _p not in sys.path:
        sys.path.insert(0, _p)

import numpy as np

import concourse.bass as bass  # noqa: F401  (bass must import before bacc)
import concourse.mybir as mybir
import concourse.tile as tile
from concourse import bacc
from concourse.bass_utils import run_bass_kernel_spmd

H = 1024
E = 8
I = 4096
TOP_K = 2
P = 128
NTOK = 512  # token tile width in phase 1 (PSUM bank = 512 fp32)
F32 = mybir.dt.float32
AF = mybir.ActivationFunctionType

_programs: dict = {}


def build_program(C: int, repeat: int = 1, dtype: str = "bf16", tg_size: int = 2) -> "bacc.Bacc":
    """One-core SPMD program: SwiGLU FFN for C tokens of one expert.

    repeat > 1 re-emits the whole computation (benchmarking aid: the HW
    time difference between repeat=2 and repeat=1 is one clean iteration).
    dtype: matmul operand precision — "bf16" (default), "f32r" (FP22
    single-pass), or "f32" (4-pass, exact). PSUM accumulation fp32 always.
    """
    assert C % P == 0
    DT = {"f32": F32, "f32r": mybir.dt.float32r, "bf16": mybir.dt.bfloat16}[dtype]
    Cb = C // P
    HB = H // P  # 8
    IB = I // P  # 32
    NH = H // NTOK  # 2
    # phase-1 token tile widths (512s + one 128-multiple remainder)
    tts = [NTOK] * (C // NTOK)
    if C % NTOK:
        tts.append(C % NTOK)
    starts = np.cumsum([0] + tts[:-1]).tolist()
    groups = [list(range(g, min(g + tg_size, len(tts))))
              for g in range(0, len(tts), tg_size)]

    nc = bacc.Bacc("TRN2", target_bir_lowering=False, debug=False, num_devices=8)
    x_d = nc.dram_tensor("xt", [H, C], DT, kind="ExternalInput")
    # w1/w3 pre-blocked on host: [i-block, p, h*128] so one i-tile is a
    # single contiguous [128, 1024] DMA
    w1_d = nc.dram_tensor("w1", [IB, P, H], DT, kind="ExternalInput")
    w3_d = nc.dram_tensor("w3", [IB, P, H], DT, kind="ExternalInput")
    w2_d = nc.dram_tensor("w2", [I, H], DT, kind="ExternalInput")
    s_d = nc.dram_tensor("st", [P, Cb], F32, kind="ExternalInput")
    # one output region per repeat so no iteration is dead code
    y_d = nc.dram_tensor("y", [repeat * C, H], F32, kind="ExternalOutput")
    # h_act scratch, [i-block, i-sub(part), c]: phase-1 stores and phase-2
    # reloads are both contiguous-row DMAs
    ha_d = nc.dram_tensor("hact", [IB, P, C], DT)

    xr = x_d.rearrange("(h p) c -> p h c", p=P)  # [128, 8, C]
    w2r = w2_d.rearrange("(i p) n -> p i n", p=P)  # [128, 32, 1024]

    with tile.TileContext(nc) as tc:
      # pools that span both phases (w2/st/h-reloads prefetched in phase 1)
      with (
          tc.tile_pool(name="w2", bufs=1) as w2_pool,
          tc.tile_pool(name="sc", bufs=1) as s_pool,
          tc.tile_pool(name="hh", bufs=64) as hh_pool,
      ):
       PRE = 4  # i-iterations of w1/w3 prefetched ahead of the w2 burst
       hh_pre: dict = {}
       for rep in range(repeat):
        # ---------------- phase 1: h = silu(x@w1) * (x@w3) ----------------
        with (
            tc.tile_pool(name=f"xt{rep}", bufs=1) as xt_pool,
            tc.tile_pool(name=f"w13{rep}", bufs=2 * PRE) as w13_pool,
            tc.tile_pool(name=f"tmp{rep}", bufs=3) as tmp_pool,
            tc.tile_pool(name=f"hst{rep}", bufs=4) as hst_pool,
            tc.tile_pool(name=f"ps1{rep}", bufs=8, space="PSUM") as ps_pool,
        ):
            # critical path first: i=0 weights + the first token groups' x
            # tiles (h-major, t interleaved — the matmul consumption order).
            # Startup dispatch alternates SP/ACT queues: DMA issue is serial
            # per engine (~0.5us/128-row descriptor), so two queues double
            # the supply rate while the PE ramps.
            _eng = [nc.sync, nc.scalar]
            _ec = [0]

            def eng():
                _ec[0] += 1
                return _eng[_ec[0] % 2]

            w13_pre = {}
            w1t = w13_pool.tile([P, HB, P], DT, tag="w13", name=f"w1_0_{rep}")
            w3t = w13_pool.tile([P, HB, P], DT, tag="w13", name=f"w3_0_{rep}")
            eng().dma_start(out=w1t[:], in_=w1_d[0])
            eng().dma_start(out=w3t[:], in_=w3_d[0])
            w13_pre[0] = (w1t, w3t)

            xts = [[None] * HB for _ in tts]

            def load_x(t, h, e=None):
                w, c0 = tts[t], starts[t]
                xtile = xt_pool.tile([P, w], DT, tag=f"x{h}_{t}", name=f"x{h}_{t}_{rep}")
                (e or nc.sync).dma_start(out=xtile[:], in_=xr[:, h, c0 : c0 + w])
                xts[t][h] = xtile

            early_t = groups[0] + (groups[1] if len(groups) > 1 else [])
            for h in range(HB):
                for t in early_t:
                    load_x(t, h, eng())
            for i in range(1, PRE):
                w1t = w13_pool.tile([P, HB, P], DT, tag="w13", name=f"w1_{i}_{rep}")
                w3t = w13_pool.tile([P, HB, P], DT, tag="w13", name=f"w3_{i}_{rep}")
                eng().dma_start(out=w1t[:], in_=w1_d[i])
                eng().dma_start(out=w3t[:], in_=w3_d[i])
                w13_pre[i] = (w1t, w3t)
            for t in range(len(tts)):
                for h in range(HB):
                    if xts[t][h] is None:
                        load_x(t, h)

            # prefetch phase-2 operands during phase 1
            if rep == 0:
                st = s_pool.tile([P, Cb], F32, tag="st", name="st")
                nc.sync.dma_start(out=st[:], in_=s_d[:])
                w2ts = []
                for i in range(IB):
                    w2t = w2_pool.tile([P, H], DT, tag=f"w2_{i}", name=f"w2_{i}")
                    nc.sync.dma_start(out=w2t[:], in_=w2r[:, i, :])
                    w2ts.append(w2t)

            for i in range(IB):
                if i in w13_pre:
                    w1t, w3t = w13_pre.pop(i)
                else:
                    w1t = w13_pool.tile([P, HB, P], DT, tag="w13", name=f"w1_{i}_{rep}")
                    w3t = w13_pool.tile([P, HB, P], DT, tag="w13", name=f"w3_{i}_{rep}")
                    nc.sync.dma_start(out=w1t[:], in_=w1_d[i])
                    nc.sync.dma_start(out=w3t[:], in_=w3_d[i])
                # token-tile groups: each weight stationary serves the whole
                # group before switching (amortizes LDWEIGHTS)
                for tg in groups:
                    p1s, p3s = {}, {}
                    for t in tg:
                        p1s[t] = ps_pool.tile([P, NTOK], F32, tag="ps", name=f"p1_{i}_{t}_{rep}")
                        p3s[t] = ps_pool.tile([P, NTOK], F32, tag="ps", name=f"p3_{i}_{t}_{rep}")
                    for h in range(HB):
                        for t in tg:
                            nc.tensor.matmul(
                                p1s[t][:, : tts[t]], w1t[:, h, :], xts[t][h][:],
                                start=(h == 0), stop=(h == HB - 1),
                            )
                    for h in range(HB):
                        for t in tg:
                            nc.tensor.matmul(
                                p3s[t][:, : tts[t]], w3t[:, h, :], xts[t][h][:],
                                start=(h == 0), stop=(h == HB - 1),
                            )
                    for t in tg:
                        w, c0, p1, p3 = tts[t], starts[t], p1s[t], p3s[t]
                        tmp = tmp_pool.tile([P, NTOK], F32, tag="tmp", name=f"tmp_{i}_{t}_{rep}")
                        nc.scalar.activation(tmp[:, :w], p1[:, :w], AF.Silu)
                        if t == 0:
                            # t=0 never touches DRAM: the epilogue writes
                            # straight into the retained phase-2 tile
                            hh = hh_pool.tile([P, NTOK], DT, tag="hh", name=f"hh_0_{i}_{rep}")
                            nc.vector.tensor_mul(hh[:, :w], tmp[:, :w], p3[:, :w])
                            hh_pre.setdefault((rep, 0), []).append(hh)
                        else:
                            hst = hst_pool.tile([P, NTOK], DT, tag="hst", name=f"h_{i}_{t}_{rep}")
                            nc.vector.tensor_mul(hst[:, :w], tmp[:, :w], p3[:, :w])
                            nc.scalar.dma_start(out=ha_d[i, :, c0 : c0 + w], in_=hst[:, :w])

        # ---------------- phase 2: y = (h @ w2) * s ----------------
        with (
            tc.tile_pool(name=f"ysb{rep}", bufs=3) as y_pool,
            tc.tile_pool(name=f"ps2{rep}", bufs=4, space="PSUM") as yps_pool,
        ):
            for t, w in enumerate(tts):
                if (rep, t) in hh_pre:
                    hhs = hh_pre.pop((rep, t))
                else:
                    hhs = []
                    for i in range(IB):
                        hh = hh_pool.tile([P, NTOK], DT, tag="hh", name=f"hh_{t}_{i}_{rep}")
                        nc.sync.dma_start(out=hh[:, :w], in_=ha_d[i, :, starts[t] : starts[t] + w])
                        hhs.append(hh)
                for k in range(w // P):
                    cb = starts[t] // P + k
                    yps = [
                        yps_pool.tile([P, NTOK], F32, tag="yps", name=f"yp_{cb}_{n}_{rep}")
                        for n in range(NH)
                    ]
                    for i in range(IB):
                        for n in range(NH):
                            nc.tensor.matmul(
                                yps[n][:],
                                hhs[i][:, k * P : (k + 1) * P],
                                w2ts[i][:, n * NTOK : (n + 1) * NTOK],
                                start=(i == 0), stop=(i == IB - 1),
                            )
                    for n in range(NH):
                        ysb = y_pool.tile([P, NTOK], F32, tag="ysb", name=f"y_{cb}_{n}_{rep}")
                        nc.scalar.activation(
                            ysb[:], yps[n][:], AF.Copy, scale=st[:, cb : cb + 1]
                        )
                        nc.sync.dma_start(
                            out=y_d[
                                rep * C + cb * P : rep * C + (cb + 1) * P,
                                n * NTOK : (n + 1) * NTOK,
                            ],
                            in_=ysb[:],
                        )

    nc.compile()
    return nc


DTYPE = os.environ.get("MOE_DTYPE", "bf16")


def get_program(C: int) -> "bacc.Bacc":
    key = (C, DTYPE)
    if key not in _programs:
        _programs[key] = build_program(C, dtype=DTYPE)
    return _programs[key]


def _gate(x: np.ndarray, gate_w: np.ndarray):
    """Top-2 routing, mirroring the jax reference (softmax -> top_k ->
    renormalize). Uses jax for bit-compatible selection when available."""
    try:
        import jax
        import jax.numpy as jnp

        logits = jnp.asarray(x) @ jnp.asarray(gate_w)
        probs = jax.nn.softmax(logits, axis=-1)
        top_vals, top_idx = jax.lax.top_k(probs, TOP_K)
        top_vals = top_vals / jnp.sum(top_vals, axis=-1, keepdims=True)
        return np.asarray(top_vals), np.asarray(top_idx)
    except Exception:
        logits = x @ gate_w
        m = logits.max(-1, keepdims=True)
        p = np.exp(logits - m)
        p /= p.sum(-1, keepdims=True)
        top_idx = np.argsort(-p, axis=-1, kind="stable")[:, :TOP_K]
        top_vals = np.take_along_axis(p, top_idx, axis=-1)
        top_vals = top_vals / top_vals.sum(-1, keepdims=True)
        return top_vals, top_idx


OVF_MAX = 32  # max token-expert pairs computed on host to save one C block


def prepare_dispatch_v2(x, gate_w):
    """Route tokens: per-expert index lists, routing weights, capacity C.

    If only a few tokens push the max expert count over a 128 boundary,
    shrink the device capacity by one block and return those tokens as
    host-overflow work ([(e, idx_array, wt_array), ...]) — every core then
    runs one fewer token block.
    """
    top_vals, top_idx = _gate(x, gate_w)
    idxs, wts = [], []
    for e in range(E):
        sel = top_idx == e  # [T, K] bool
        mask = sel.any(axis=-1)
        idx_e = np.nonzero(mask)[0]
        w_e = np.where(sel[idx_e, 0], top_vals[idx_e, 0], top_vals[idx_e, 1])
        idxs.append(idx_e)
        wts.append(w_e.astype(np.float32))
    max_cnt = max(len(ix) for ix in idxs)
    C = max(NTOK, -(-max_cnt // P) * P)
    ovf = []
    C1 = C - P
    if C1 >= NTOK:
        n_over = sum(max(0, len(ix) - C1) for ix in idxs)
        if 0 < n_over <= OVF_MAX:
            for e in range(E):
                if len(idxs[e]) > C1:
                    ovf.append((e, idxs[e][C1:], wts[e][C1:]))
                    idxs[e] = idxs[e][:C1]
                    wts[e] = wts[e][:C1]
            C = C1
    return idxs, wts, C, ovf


def prepare_dispatch(x, gate_w):
    idxs, wts, C, _ = prepare_dispatch_v2(x, gate_w)
    return idxs, wts, C


def _block_w13(w):
    """[H, I] -> [IB, P, H]: w_blocked[i, p, h*P + c] = w[h*P + p, i*P + c]."""
    HB, IB = H // P, I // P
    return np.ascontiguousarray(
        w.reshape(HB, P, IB, P).transpose(2, 1, 0, 3).reshape(IB, P, H)
    )


def make_in_maps(x, w1, w3, w2, idxs, wts, C, dtype=None):
    dtype = dtype or DTYPE
    if dtype == "bf16":
        import ml_dtypes
        npdt = ml_dtypes.bfloat16
    else:
        npdt = np.float32
    Cb = C // P
    in_maps = []
    for e in range(E):
        cnt = len(idxs[e])
        x_pad = np.zeros((C, H), np.float32)
        x_pad[:cnt] = x[idxs[e]]
        s_pad = np.zeros(C, np.float32)
        s_pad[:cnt] = wts[e]
        in_maps.append(
            {
                "xt": np.ascontiguousarray(x_pad.T).astype(npdt),
                "w1": _block_w13(np.asarray(w1[e], dtype=np.float32)).astype(npdt),
                "w3": _block_w13(np.asarray(w3[e], dtype=np.float32)).astype(npdt),
                "w2": np.ascontiguousarray(np.asarray(w2[e], dtype=np.float32)).astype(npdt),
                "st": np.ascontiguousarray(s_pad.reshape(Cb, P).T),
            }
        )
    return in_maps


def combine(results, idxs, T):
    out = np.zeros((T, H), np.float32)
    for e in range(E):
        cnt = len(idxs[e])
        out[idxs[e]] += results[e]["y"][:cnt]
    return out


def kernel(hidden_states, gate_w, w1, w3, w2):
    B, S, Hh = hidden_states.shape
    assert Hh == H
    x = np.ascontiguousarray(hidden_states.reshape(-1, H), dtype=np.float32)
    T = x.shape[0]

    idxs, wts, C, ovf = prepare_dispatch_v2(x, gate_w)
    nc = get_program(C)
    in_maps = make_in_maps(x, w1, w3, w2, idxs, wts, C)
    res = run_bass_kernel_spmd(nc, in_maps, list(range(E)))
    out = combine(res.results, idxs, T)
    for e, t_idx, t_w in ovf:  # host-side capacity-overflow tokens
        xe = x[t_idx]
        w1e = np.asarray(w1[e], np.float32)
        w3e = np.asarray(w3[e], np.float32)
        w2e = np.asarray(w2[e], np.float32)
        a = xe @ w1e
        h = (a / (1.0 + np.exp(-a))) * (xe @ w3e)
        out[t_idx] += (h @ w2e) * t_w[:, None]
    return out.reshape(B, S, H)
